# revision 47
# baseline (speedup 1.0000x reference)
"""Trainium2 Bass kernel for nn_BaichuanAttention (ALiBi attention + KV cache).

Head-parallel across 8 NeuronCores (4 heads/core). Per core:
  - QKV projection in transposed layout (fp16 matmuls, fp32 PSUM)
  - attention with transposed scores [keys, queries]:
      * ALiBi per-key term folded into the ACT exp() per-partition bias
      * ALiBi per-query term + causal mask applied via one DVE add
        (the per-query term is softmax-shift-invariant; it only fixes fp range)
      * softmax denominator via ones-matmul on the PE
  - PV accumulation, normalize via outer-product broadcast, o_proj partial
Host: shard/transpose/cast inputs, sum o_proj partials across cores.
"""
import os
import sys

import numpy as np

for _p in ("/opt/trn_rl_repo",):
    if os.path.isdir(_p) and _p not in sys.path:
        sys.path.insert(0, _p)

import concourse.bass as bass
import concourse.mybir as mybir
import concourse.tile as tile
from concourse import bacc
from concourse.bass_utils import run_bass_kernel_spmd
from concourse.masks import make_identity

F32 = mybir.dt.float32
F32R = mybir.dt.float32r
F16 = mybir.dt.float16

B, S, D, H, HD, HIST = 1, 2048, 4096, 32, 128, 1024
T = HIST + S
NCORES = 8
HPC = H // NCORES          # heads per core
FPC = HPC * HD             # 512 features per core per section
NST = S // 512             # 4 query supertiles
W = 512                    # supertile width
NKT = T // 128             # 24 key tiles
NKC = D // 128             # 32 contraction chunks for QKV
NEG = -1.0e30


def _alibi_slopes(n_heads: int) -> np.ndarray:
    def pow2_slopes(m):
        start = 2.0 ** (-(2.0 ** -(np.log2(m) - 3)))
        return start * (start ** np.arange(m))
    if np.log2(n_heads).is_integer():
        return pow2_slopes(n_heads).astype(np.float32)
    m = 2 ** int(np.floor(np.log2(n_heads)))
    base = pow2_slopes(m)
    extra = pow2_slopes(2 * m)[0::2][: n_heads - m]
    return np.concatenate([base, extra]).astype(np.float32)


def _emit(nc):
    """Emit the whole per-core program under a TileContext."""
    xT_d = nc.dram_tensor("xT", [D, S], F16, kind="ExternalInput").ap()
    wT_d = nc.dram_tensor("wT", [D, 3 * FPC], F16, kind="ExternalInput").ap()
    pkT_d = nc.dram_tensor("pkT", [HPC, HD, HIST], F16, kind="ExternalInput").ap()
    pv_d = nc.dram_tensor("pv", [HPC, HIST, HD], F16, kind="ExternalInput").ap()
    opT_d = nc.dram_tensor("opT", [FPC, D], F16, kind="ExternalInput").ap()
    ab_d = nc.dram_tensor("abias", [128, HPC * NST * NKT], F32, kind="ExternalInput").ap()
    dvb_d = nc.dram_tensor("dvb", [HPC, 128, W], F32, kind="ExternalInput").ap()
    mk_d = nc.dram_tensor("mk", [1, 128, 128], mybir.dt.bfloat16, kind="ExternalInput").ap()
    out_d = nc.dram_tensor("outp", [S, D], F32, kind="ExternalOutput").ap()

    with tile.TileContext(nc) as tc:
        with (
            tc.tile_pool(name="persist", bufs=1) as pers,
            tc.tile_pool(name="xslab", bufs=9) as xpool,
            tc.tile_pool(name="wts", bufs=36) as wpool,
            tc.tile_pool(name="s1", bufs=3) as s1pool,
            tc.tile_pool(name="pp", bufs=3) as ppool,
            tc.tile_pool(name="small", bufs=1) as smallpool,
            tc.tile_pool(name="ob", bufs=3) as obpool,
            tc.tile_pool(name="at", bufs=2) as atpool,
            tc.tile_pool(name="ps_qkv", bufs=2, space="PSUM") as ps_qkv,
            tc.tile_pool(name="ps_s", bufs=2, space="PSUM") as ps_s,
            tc.tile_pool(name="ps_o", bufs=1, space="PSUM") as ps_o,
            tc.tile_pool(name="ps_d", bufs=1, space="PSUM") as ps_d,
            tc.tile_pool(name="ps_sh", bufs=2, space="PSUM") as ps_sh,
        ):
            # ---- persistent SBUF tensors ----
            qkT = [pers.tile([128, S], F16, tag=f"qkT{f}", bufs=1, name=f"qkT{f}") for f in range(8)]
            vT = [pers.tile([128, S], F16, tag=f"vT{h}", bufs=1, name=f"vT{h}") for h in range(HPC)]
            pk_sb = [pers.tile([128, HIST], F16, tag=f"pk{h}", bufs=1, name=f"pk{h}") for h in range(HPC)]
            pv_sb = [pers.tile([128, HIST], F16, tag=f"pvs{h}", bufs=1, name=f"pvs{h}") for h in range(HPC)]
            attn_tiles = {}
            ab_sb = pers.tile([128, HPC * NST * NKT], F32, tag="abias", bufs=1)
            ident = pers.tile([128, 128], F16, tag="ident", bufs=1)
            ones16 = pers.tile([128, 1], F16, tag="ones16", bufs=1)

            make_identity(nc, ident)
            nc.any.memset(ones16[:], 1.0)
            nc.gpsimd.dma_start(ab_sb[:], ab_d[:])
            for h in range(HPC):
                nc.gpsimd.dma_start(pk_sb[h][:], pkT_d[h])
                nc.gpsimd.dma_start(
                    pv_sb[h].rearrange("p (c d) -> p c d", c=HIST // 128),
                    pv_d[h].rearrange("(c p) d -> p c d", p=128),
                )

            # per-head row bias [128, W] f32 (broadcast rows) + masks
            rowt = []
            maskt = []
            for h in range(HPC):
                t = pers.tile([128, W], F32, tag=f"row{h}", bufs=1, name=f"row{h}")
                nc.gpsimd.dma_start(t[:], dvb_d[h])
                rowt.append(t)
            tri = pers.tile([128, 128], mybir.dt.bfloat16, tag="tri", bufs=1)
            nc.gpsimd.dma_start(tri[:], mk_d[0])

            opw = {}
            for n in range(8):
                for h in range(HPC):
                    t = pers.tile([128, W], F16, tag=f"opw{n}_{h}", bufs=1,
                                  name=f"opw{n}_{h}")
                    nc.gpsimd.dma_start(
                        t[:],
                        opT_d[h * 128:(h + 1) * 128, n * W:(n + 1) * W],
                    )
                    opw[(n, h)] = t

            def qkv_supertile(sc):
                """QKV projection for query supertile sc (12 f-chunks x 32 k).

                W loads are batched [128, 512] (4 f-chunks per DMA) and x
                loads [128, 4, 512] (4 k-chunks per DMA) to cut DMA count.
                """
                xk = [None] * (NKC // 4)

                def load_xk(c):
                    t = xpool.tile([128, 4, W], F16, tag="xk", name=f"xk{c}")
                    xs = xT_d[c * 512:(c + 1) * 512, sc * W:(sc + 1) * W]
                    nc.sync.dma_start(t[:], xs.rearrange("(j p) s -> p j s", p=128))
                    xk[c] = t

                for fcg in range(3):
                    wtg = []
                    for k in range(NKC):
                        if fcg == 0 and k % 4 == 0:
                            load_xk(k // 4)
                        t = wpool.tile([128, W], F16, tag="wtg", name=f"wtg{k}")
                        nc.sync.dma_start(
                            t[:],
                            wT_d[k * 128:(k + 1) * 128,
                                 fcg * 512:(fcg + 1) * 512],
                        )
                        wtg.append(t)
                    for fi in range(4):
                        fc = fcg * 4 + fi
                        psum = ps_qkv.tile([128, W], F32, name="qkvps")
                        for k in range(NKC):
                            nc.tensor.matmul(
                                psum[:],
                                wtg[k][:, fi * 128:(fi + 1) * 128],
                                xk[k // 4][:, k % 4, :],
                                start=(k == 0), stop=(k == NKC - 1),
                            )
                        if fc < 8:
                            dst = qkT[fc]
                        else:
                            dst = vT[fc - 8]
                        nc.any.tensor_copy(dst[:, sc * W:(sc + 1) * W], psum[:])

            def vtrans_supertile(sc):
                """Transpose new-v chunks for supertile sc: vT -> vn."""
                for h in range(HPC):
                    for t4 in range(4):
                        t = sc * 4 + t4
                        tp = ps_sh.tile([128, W], F16, tag="sh", name="vtps")
                        nc.tensor.transpose(
                            tp[:, :128], vT[h][:, t * 128:(t + 1) * 128], ident[:]
                        )
                        nc.any.tensor_copy(
                            vT[h][:, t * 128:(t + 1) * 128], tp[:, :128]
                        )

            def attention(h, s):
                """One head, one query supertile: scores^T, exp, PV, denom, norm."""
                nvis = 12 + 4 * s
                o_ps = ps_o.tile([128, W], F32, name="ops")
                d_ps = ps_d.tile([1, W], F32, name="dps")
                for j in range(nvis):
                    m = j - (8 + 4 * s)
                    off = 0 if m < 0 else 128 * m
                    nv = W - off
                    if s >= 2 and j % 2 == 1:
                        sp = ps_qkv.tile([128, W], F32, name="qkvps")
                    else:
                        sp = ps_s.tile([128, W], F32, tag="sps", name="sps")
                    if j < 8:
                        kt = pk_sb[h][:, j * 128:(j + 1) * 128]
                    else:
                        kt = qkT[4 + h][:, (j - 8) * 128:(j - 7) * 128]
                    nc.tensor.matmul(
                        sp[:, :nv], kt,
                        qkT[h][:, s * W + off:(s + 1) * W],
                        start=True, stop=True,
                    )
                    s1 = s1pool.tile([128, W], F32, tag="s1", name="s1")
                    nc.vector.tensor_add(
                        s1[:, :nv], sp[:, :nv], rowt[h][:, off:]
                    )
                    if m >= 0:
                        nc.vector.tensor_add(s1[:, :128], s1[:, :128], tri[:])
                    p = ppool.tile([128, W], F16, tag="p", name="p")
                    col = (h * NST + s) * NKT + j
                    nc.scalar.activation(
                        p[:, :nv], s1[:, :nv],
                        mybir.ActivationFunctionType.Exp,
                        bias=ab_sb[:, col:col + 1],
                    )
                    if j < 8:
                        vt = pv_sb[h][:, j * 128:(j + 1) * 128]
                    else:
                        vt = vT[h][:, (j - 8) * 128:(j - 7) * 128]
                    nc.tensor.matmul(
                        o_ps[:, off:], vt, p[:, :nv],
                        start=(j == 0), stop=(j == nvis - 1),
                    )
                    nc.tensor.matmul(
                        d_ps[:, off:], ones16[:], p[:, :nv],
                        start=(j == 0), stop=(j == nvis - 1),
                    )
                # normalize: attnT[:, s] = o_ps * (1/den) broadcast over partitions
                denr = smallpool.tile([1, W], F32, tag="denr", name="denr")
                nc.vector.reciprocal(denr[:], d_ps[:])
                bb = s1pool.tile([128, W], F32, tag="s1", name="bb")
                nc.gpsimd.partition_broadcast(bb[:], denr[:])
                at = atpool.tile([128, W], F16, tag=f"at{h}", name=f"at{h}_{s}")
                attn_tiles[(h, s)] = at
                nc.vector.tensor_mul(at[:], o_ps[:], bb[:])

            def oproj_supertile(s):
                """o_proj partial rows for queries in supertile s."""
                for n in range(8):
                    opn = [opw[(n, h)] for h in range(HPC)]
                    for m4 in range(4):
                        m = s * 4 + m4
                        if s == 3 and (n * 4 + m4) % 2 == 1:
                            psum = ps_s.tile([128, W], F32, tag="sps", name="sps")
                        else:
                            psum = ps_sh.tile([128, W], F32, tag="sh", name="shps")
                        for h in range(HPC):
                            nc.tensor.matmul(
                                psum[:],
                                attn_tiles[(h, s)][:, m4 * 128:(m4 + 1) * 128],
                                opn[h][:],
                                start=(h == 0), stop=(h == HPC - 1),
                            )
                        ob = obpool.tile([128, W], F32, tag="ob", name="ob")
                        nc.any.tensor_copy(ob[:], psum[:])
                        nc.sync.dma_start(
                            out_d[m * 128:(m + 1) * 128, n * W:(n + 1) * W], ob[:]
                        )

            # ---- pipelined emission order ----
            qkv_supertile(0)
            vtrans_supertile(0)
            qkv_supertile(1)
            vtrans_supertile(1)
            for h in range(HPC):
                attention(h, 0)
            qkv_supertile(2)
            vtrans_supertile(2)
            oproj_supertile(0)
            for h in range(HPC):
                attention(h, 1)
            qkv_supertile(3)
            vtrans_supertile(3)
            oproj_supertile(1)
            for h in range(HPC):
                attention(h, 2)
            oproj_supertile(2)
            for h in range(HPC):
                attention(h, 3)
            oproj_supertile(3)

    return nc


_CACHE = {}


def _build():
    if "nc" not in _CACHE:
        nc = bacc.Bacc(
            trn_type="TRN2", target_bir_lowering=False, debug=False,
            num_devices=NCORES,
        )
        _emit(nc)
        nc.compile()
        _CACHE["nc"] = nc
    return _CACHE["nc"]


def _host_prep(hidden_states, past_key, past_value, W_pack_w, o_proj_w):
    x = np.asarray(hidden_states, np.float32).reshape(S, D)
    pk = np.asarray(past_key, np.float32).reshape(H, HIST, HD)
    pv = np.asarray(past_value, np.float32).reshape(H, HIST, HD)
    Wp = np.asarray(W_pack_w, np.float32)
    Wo = np.asarray(o_proj_w, np.float32)
    slopes = _alibi_slopes(H)

    xT = np.ascontiguousarray(x.T).astype(np.float16)
    scale = np.float32(1.0 / np.sqrt(HD))
    kk = np.arange(128, dtype=np.float32)
    ii = np.arange(W, dtype=np.float32)

    in_maps = []
    for c in range(NCORES):
        h0 = c * HPC
        f0 = h0 * HD
        Wq = Wp[f0:f0 + FPC] * scale
        Wk = Wp[D + f0:D + f0 + FPC]
        Wv = Wp[2 * D + f0:2 * D + f0 + FPC]
        wT = np.ascontiguousarray(
            np.concatenate([Wq, Wk, Wv], 0).T
        ).astype(np.float16)
        pkT = np.ascontiguousarray(
            pk[h0:h0 + HPC].transpose(0, 2, 1)
        ).astype(np.float16)
        pvc = np.ascontiguousarray(pv[h0:h0 + HPC]).astype(np.float16)
        opT = np.ascontiguousarray(
            Wo[:, f0:f0 + FPC].T
        ).astype(np.float16)
        sl = slopes[h0:h0 + HPC]

        ab = np.zeros((HPC, NST, NKT, 128), np.float32)
        for h in range(HPC):
            for s in range(NST):
                for j in range(NKT):
                    ab[h, s, j] = sl[h] * (128 * j + kk - HIST - W * s)
        ab_sb = np.ascontiguousarray(
            ab.reshape(HPC * NST * NKT, 128).T
        )

        import ml_dtypes
        dvb = np.zeros((HPC, 128, W), np.float32)
        for h in range(HPC):
            dvb[h] = (-sl[h] * ii)[None, :]
        mk = np.where(ii[None, None, :128] >= kk[None, :, None], 0.0,
                      NEG).astype(ml_dtypes.bfloat16)
        in_maps.append({
            "xT": xT, "wT": wT, "pkT": pkT, "pv": pvc, "opT": opT,
            "abias": ab_sb, "dvb": dvb, "mk": mk,
        })
    return in_maps


def kernel(hidden_states, past_key, past_value, W_pack_w, o_proj_w):
    nc = _build()
    in_maps = _host_prep(hidden_states, past_key, past_value, W_pack_w, o_proj_w)
    res = run_bass_kernel_spmd(nc, in_maps, list(range(NCORES)))
    out = np.zeros((S, D), np.float64)
    for c in range(NCORES):
        out += res.results[c]["outp"].astype(np.float64)
    return out.astype(np.float32).reshape(B, S, D)
